# revision 23
# baseline (speedup 1.0000x reference)
"""Trainium2 Bass kernel for nn_CryptoGNN (2-layer GCN + pooled heads).

Math (validated against the reference):
  With A = normalized adjacency (incl. self loops), P = [B,N] pooling matrix:
    r_i = sum_{real e: j->i} dis_j x_j + dis_i x_i          (6 feats)
    z_i = r_i @ W1 + sqrt(deg_i) b1        ->  h1_i = dis_i * relu(z_i)
    G   = [PA; P] @ h1   ([128,128]; PA = P@A dense, built on host)
  Host head finishes:  P@h2 = (PA@h1)@W2 + cnt*b2 + P@h1, then the tiny
  [64,*] MLPs (microseconds, numpy).

Sharding: nodes split into 8 contiguous dst-shards of 12544, one NeuronCore
each.  No collectives: the host sums the 8 partial [128,128] G outputs.

Per-core device pipeline (4 dst-chunks, sized small/big/big/small so the
first table DMA and the last chunk's tail are short):
  * per (chunk, src-group) compacted feature table [128, NEC_c] f32 -- only
    src columns with >=1 edge into the (core, chunk) are shipped (~20% of
    12.5k), dead rows host-zeroed, dis[src] pre-folded.
  * GPSIMD ap_gather #1: per-group dst-sorted edge stream of src columns
  * fp32 prefix scan along the stream (DVE tensor_tensor_scan)
  * GPSIMD ap_gather #2 of per-dst boundary prefix values
  * DVE shifted difference -> per-dst segment sums, downcast to bf16 dt
    (emitted in 1024-col pieces so mm batches start early)
  * mm1 (bf16): z tile = dt[:,tile]^T @ (selp@W1) + aug7[:,tile]^T @ [W1;b1]
    -- the 8-group fold is pre-composed into the host weight wp, so there
    is no separate fold stage or PSUM copy.  aug7 rows = dis*x self-loop
    feats + sqrt(deg) (for b1), host-computed.
  * relu in 8-tile batches (Act) -> bf16 h1
  * mm2 (bf16): G += papt[:,tile]^T @ h1; papt is bf16 with dis[dst]
    pre-folded, stored pair-interleaved ([49,128,256]) so its DMA runs at
    full rate (512B contiguous runs).

GPSIMD is the bottleneck engine (~32us busy); the 4-chunk structure keeps
it streaming continuously while DVE scans/diffs and PE/Act run earlier
chunks' matmuls.  Index tiles keep the boundary half 4-byte aligned
(JWC_c % 32 == 0) -- the gather ucode requires it (CoreSim does not
model this; misalignment silently corrupts on silicon).
"""

import sys

if "/opt/trn_rl_repo" not in sys.path:
    sys.path.insert(0, "/opt/trn_rl_repo")

import numpy as np

N = 100000
E = 600000
B = 64
IN = 6
H = 128

NSHARD = 12544            # nodes per core shard
NG = 8                    # src groups (= table partition groups)
NPAD = NSHARD * NG        # 100352
NT = NSHARD // 128        # 98 node tiles per shard
P128 = 128

NCHUNK = 4
CHUNK_TILES = (8, 34, 34, 22)
CHUNK_NODES = tuple(t * 128 for t in CHUNK_TILES)          # 1024 4352 4352 2816
CHUNK_OFF = (0, 1024, 5376, 9728)
CHUNK_TILE0 = (0, 8, 42, 76)
NBCS = tuple(((n + 1 + 15) // 16) * 16 for n in CHUNK_NODES)

_compiled = {}


def _r16(v):
    return ((int(v) + 15) // 16) * 16


def _r32(v):
    # multiples of 32 keep the bidx half of the concatenated int16 index
    # tile 4-byte aligned for the GPSIMD gather ucode
    return ((int(v) + 31) // 32) * 32


def _build_nc(shape_key):
    import concourse.bacc as bacc
    import concourse.mybir as mybir
    from concourse import tile

    NECS, JWCS = shape_key
    f32 = mybir.dt.float32
    bf16 = mybir.dt.bfloat16
    i16 = mybir.dt.int16

    NEC_MX = max(NECS)
    JWC_MX = max(JWCS)
    NBC_MX = max(NBCS)

    nc = bacc.Bacc("TRN2", target_bir_lowering=False, debug=False)

    tabs = [nc.declare_dram_parameter(f"tab{c}", [P128, NECS[c]], f32,
                                      isOutput=False) for c in range(NCHUNK)]
    gbs = [nc.declare_dram_parameter(f"gb{c}", [P128, (JWCS[c] + NBCS[c]) // 16],
                                     i16, isOutput=False) for c in range(NCHUNK)]
    aug7 = nc.declare_dram_parameter("aug7", [7, NSHARD], bf16, isOutput=False)
    w1a = nc.declare_dram_parameter("w1a", [7, H], bf16, isOutput=False)
    wp = nc.declare_dram_parameter("wp", [P128, H], bf16, isOutput=False)
    papt = nc.declare_dram_parameter("papt", [NT // 2, P128, 256], bf16,
                                     isOutput=False)
    gout = nc.declare_dram_parameter("gout", [P128, P128], f32, isOutput=True)

    with tile.TileContext(nc) as tc:
        with (
            tc.tile_pool(name="small", bufs=1) as small,
            tc.tile_pool(name="tabp", bufs=3) as tabp,
            tc.tile_pool(name="idxp", bufs=4) as idxp,
            tc.tile_pool(name="gathp", bufs=3) as gathp,
            tc.tile_pool(name="bndp", bufs=2) as bndp,
            tc.tile_pool(name="dtp", bufs=2) as dtp,
            tc.tile_pool(name="h1p", bufs=3) as h1p,
            tc.tile_pool(name="ps1", bufs=2, space="PSUM") as ps1p,
            tc.tile_pool(name="psG", bufs=1, space="PSUM") as psGp,
        ):
            # ---- constants (DMAs issued later, after the first tables) ----
            # wp = selp @ W1 (host-precomposed group-replicated W1 rows) so
            # mm1 contracts the bf16 diff output directly -- no fold stage.
            wp_t = small.tile([P128, H], bf16)
            w1_t = small.tile([7, H], bf16)
            # aug rows: 0-5 dis*x self-loop feats, 6 sqrt(deg)
            axT = small.tile([7, NSHARD], bf16)

            def dma_consts():
                nc.sync.dma_start(out=wp_t[:], in_=wp[:])
                nc.sync.dma_start(out=w1_t[:], in_=w1a[:])
                nc.sync.dma_start(out=axT[:], in_=aug7[:])

            # whole papt in SBUF (bf16, interleaved pairs), 4 slab DMAs
            papt_sb = small.tile([P128, NT // 2 * 256], bf16)
            SLABS = [(0, 13), (13, 12), (25, 12), (37, 12)]

            G_ps = psGp.tile([P128, P128], f32, tag="G")

            tab_t = [None] * NCHUNK
            dt_t = [None] * NCHUNK
            gb_t = [None] * NCHUNK
            gath_t = [None] * NCHUNK
            bnd_t = [None] * NCHUNK

            def dma_tab(c):
                tab_t[c] = tabp.tile([P128, NEC_MX], f32, tag="tab",
                                     name=f"tab_t{c}")
                nc.sync.dma_start(out=tab_t[c][:, 0:NECS[c]], in_=tabs[c][:])
                gb_t[c] = idxp.tile([P128, (JWCS[c] + NBCS[c]) // 16], i16,
                                    tag=f"gb{c}", name=f"gb_t{c}")
                nc.sync.dma_start(out=gb_t[c][:], in_=gbs[c][:])

            def dma_slab(s):
                u0, nu = SLABS[s]
                nc.sync.dma_start(
                    out=papt_sb[:, u0 * 256:(u0 + nu) * 256].rearrange(
                        "p (u j) -> p u j", j=256),
                    in_=papt[u0:u0 + nu].rearrange("u p j -> p u j"),
                )

            def g1(c):
                gath_t[c] = gathp.tile([P128, JWC_MX], f32, tag="gath",
                                       name=f"gath_t{c}")
                nc.gpsimd.ap_gather(
                    out_ap=gath_t[c][:, 0:JWCS[c]],
                    in_ap=tab_t[c][:, 0:NECS[c]],
                    idxs_ap=gb_t[c][:, :JWCS[c] // 16],
                    channels=P128, num_elems=NECS[c], d=1, num_idxs=JWCS[c],
                )

            def scan(c):
                g = gath_t[c][:, 0:JWCS[c]]
                nc.vector.tensor_tensor_scan(
                    out=g, data0=g, data1=g, initial=0.0,
                    op0=mybir.AluOpType.add, op1=mybir.AluOpType.bypass,
                )

            def g2(c):
                bnd_t[c] = bndp.tile([P128, NBC_MX], f32, tag="bnd",
                                     name=f"bnd_t{c}")
                nc.gpsimd.ap_gather(
                    out_ap=bnd_t[c][:, 0:NBCS[c]],
                    in_ap=gath_t[c][:, 0:JWCS[c]],
                    idxs_ap=gb_t[c][:, JWCS[c] // 16:],
                    channels=P128, num_elems=JWCS[c], d=1, num_idxs=NBCS[c],
                )

            def diff(c):
                # dt = bnd[:,1:] - bnd[:,:-1] downcast to bf16, emitted in
                # 1024-col pieces so the first mm batch starts early
                cn = CHUNK_NODES[c]
                step = 1024
                dt_t[c] = dtp.tile([P128, max(CHUNK_NODES)], bf16, tag="dt",
                                   name=f"dt_t{c}")
                for p0 in range(0, cn, step):
                    sz = min(step, cn - p0)
                    nc.vector.tensor_tensor(
                        out=dt_t[c][:, p0:p0 + sz],
                        in0=bnd_t[c][:, p0 + 1:p0 + 1 + sz],
                        in1=bnd_t[c][:, p0:p0 + sz],
                        op=mybir.AluOpType.subtract,
                    )

            pending = []   # (h1_t, first_tile, ntiles) awaiting mm2

            def flush_mm2():
                h1_t, t0, m = pending.pop(0)
                for u in range(m):
                    t = t0 + u
                    uu, vv = divmod(t, 2)
                    pcol = uu * 256 + vv * 128
                    nc.tensor.matmul(
                        out=G_ps[:],
                        lhsT=papt_sb[:, pcol:pcol + 128],
                        rhs=h1_t[:, u * H:(u + 1) * H],
                        start=(t == 0), stop=(t == NT - 1),
                    )

            def mm(c):
                # z tile = dt[:,tile]^T @ wp + aug[:,tile]^T @ [W1;b1]
                # (PSUM accumulate); relu in 8-tile batches (alternating
                # Act/DVE); mm2 is software-pipelined one batch behind so
                # the in-order PE queue never stalls on a relu.
                off = CHUNK_OFF[c]
                dt = dt_t[c]
                tc0 = CHUNK_TILE0[c]
                qb = 8
                for q in range(0, CHUNK_TILES[c], qb):
                    m = min(qb, CHUNK_TILES[c] - q)
                    ps1_t = ps1p.tile([P128, 8 * H], f32, tag="ps1")
                    for u in range(m):
                        lo = (q + u) * 128
                        nc.tensor.matmul(
                            out=ps1_t[:, u * H:(u + 1) * H],
                            lhsT=dt[:, lo:lo + 128],
                            rhs=wp_t[:],
                            start=True, stop=False,
                        )
                        nc.tensor.matmul(
                            out=ps1_t[:, u * H:(u + 1) * H],
                            lhsT=axT[:, off + lo:off + lo + 128],
                            rhs=w1_t[:],
                            start=False, stop=True,
                        )
                    while pending:
                        flush_mm2()
                    h1_t = h1p.tile([P128, 8 * H], bf16, tag="h1")
                    nc.scalar.activation(
                        out=h1_t[:, :m * H], in_=ps1_t[:, :m * H],
                        func=mybir.ActivationFunctionType.Relu)
                    pending.append((h1_t, tc0 + q, m))

            # ---- pipelined emission (deps drive the schedule; tables
            # first so GPSIMD starts early and never starves) ----
            dma_tab(0)
            dma_tab(1)
            dma_consts()
            g1(0)
            dma_tab(2)
            scan(0)
            g1(1)
            dma_tab(3)
            g2(0)
            scan(1)
            diff(0)
            dma_slab(0)
            mm(0)
            g1(2)
            dma_slab(1)
            g2(1)
            scan(2)
            diff(1)
            mm(1)
            g1(3)
            dma_slab(2)
            dma_slab(3)
            g2(2)
            scan(3)
            diff(2)
            mm(2)
            g2(3)
            diff(3)
            mm(3)
            while pending:
                flush_mm2()

            G_sb = small.tile([P128, P128], f32)
            nc.scalar.activation(
                out=G_sb[:], in_=G_ps[:],
                func=mybir.ActivationFunctionType.Copy,
            )
            nc.sync.dma_start(out=gout[:], in_=G_sb[:])

    nc.compile()
    return nc


def _preprocess(x, edge_index, batch_idx):
    """Host-side integer/structure preprocessing -> per-core input maps
    (minus weights) + head constants."""
    import ml_dtypes

    bf = ml_dtypes.bfloat16

    src = np.asarray(edge_index[0], dtype=np.int64)
    dst = np.asarray(edge_index[1], dtype=np.int64)

    deg = (np.bincount(dst, minlength=N) + 1.0).astype(np.float32)
    dis = (1.0 / np.sqrt(deg)).astype(np.float32)
    sqdeg = np.sqrt(deg).astype(np.float32)

    bi = np.asarray(batch_idx, dtype=np.int64)
    cnt = np.bincount(bi, minlength=B).astype(np.float32)

    dis_pad = np.zeros(NPAD, np.float32)
    dis_pad[:N] = dis

    # dense PA = P @ A  [B, NPAD] (incl. self loops), exact fp64 accumulate
    loop = np.arange(N, dtype=np.int64)
    src2 = np.concatenate([src, loop])
    dst2 = np.concatenate([dst, loop])
    w = (dis[src2] * dis[dst2]).astype(np.float64)
    flat = bi[dst2] * NPAD + src2
    PA = np.bincount(flat, weights=w, minlength=B * NPAD)
    PA = PA.reshape(B, NPAD).astype(np.float32)
    Pm = np.zeros((B, NPAD), np.float32)
    Pm[bi, np.arange(N)] = 1.0
    papt_full = (np.concatenate([PA, Pm], axis=0)
                 * dis_pad[None, :]).T.copy()      # [NPAD, 128]

    # per-core papt: bf16, DMA-interleaved [49, 128, 256]
    papt_cores = []
    for k in range(NG):
        pk = papt_full[k * NSHARD:(k + 1) * NSHARD].astype(bf)
        pk = pk.reshape(NT // 2, 2, 128, 128).transpose(0, 2, 1, 3)
        papt_cores.append(np.ascontiguousarray(pk.reshape(NT // 2, 128, 256)))

    # per-core aug rows: 0-5 dis*x, 6 sqrt(deg)
    x_np = np.asarray(x, dtype=np.float32)
    selfx = (x_np * dis[:, None]).astype(np.float32)   # [N, 6]
    aug_cores = []
    for k in range(NG):
        a = np.zeros((7, NSHARD), np.float32)
        n0 = k * NSHARD
        n1 = min(n0 + NSHARD, N)
        a[0:6, 0:n1 - n0] = selfx[n0:n1].T
        a[6, 0:n1 - n0] = sqdeg[n0:n1]
        aug_cores.append(a.astype(bf))

    # ---- edge partitioning: (core, chunk, group), dst-sorted ----
    core = dst // NSHARD
    dstl = dst - core * NSHARD
    ch_off = np.asarray(CHUNK_OFF + (NSHARD,), np.int64)
    chunk = np.searchsorted(ch_off, dstl, side="right") - 1
    dstc = dstl - ch_off[chunk]
    grp = src // NSHARD
    srcl = src - grp * NSHARD

    cell = (core * NCHUNK + chunk) * NG + grp          # [0, 256)
    key = cell * 8192 + dstc                           # dstc < 8192
    order = np.argsort(key, kind="stable")
    cell_s = cell[order]
    srcl_s = srcl[order]
    dstc_s = dstc[order]
    counts = np.bincount(cell_s, minlength=NG * NCHUNK * NG)
    starts = np.zeros(NG * NCHUNK * NG + 1, np.int64)
    np.cumsum(counts, out=starts[1:])

    # pass 1: per-cell compaction
    uniqs = {}
    invs = {}
    max_used = [0] * NCHUNK
    max_cnt = [0] * NCHUNK
    for k in range(NG):
        for c in range(NCHUNK):
            for g in range(NG):
                ci = (k * NCHUNK + c) * NG + g
                s0, s1 = starts[ci], starts[ci + 1]
                u, inv = np.unique(srcl_s[s0:s1], return_inverse=True)
                uniqs[(k, c, g)] = u
                invs[(k, c, g)] = inv
                max_used[c] = max(max_used[c], len(u))
                max_cnt[c] = max(max_cnt[c], s1 - s0)

    NECS = tuple(_r16(mu + 16) for mu in max_used)     # last col(s) stay zero
    JWCS = tuple(_r32(mc + 2) for mc in max_cnt)

    tabs_all = [[None] * NCHUNK for _ in range(NG)]
    gbs_all = [[None] * NCHUNK for _ in range(NG)]
    for k in range(NG):
        for c in range(NCHUNK):
            NEC, JWC, NBC = NECS[c], JWCS[c], NBCS[c]
            zcol = NEC - 1
            tab = np.zeros((P128, NEC), np.float32)
            gidx = np.full((P128, JWC // 16), zcol, np.int16)
            bidx = np.zeros((P128, NBC // 16), np.int16)
            for g in range(NG):
                ci = (k * NCHUNK + c) * NG + g
                s0, s1 = starts[ci], starts[ci + 1]
                u = uniqs[(k, c, g)]
                inv = invs[(k, c, g)]
                nu = len(u)
                if nu:
                    gl = g * NSHARD + u
                    tab[16 * g:16 * g + 6, :nu] = (x_np[gl] * dis[gl, None]).T
                stream = np.full(JWC, zcol, np.int64)
                stream[1:1 + (s1 - s0)] = inv
                gidx[16 * g:16 * (g + 1)] = (
                    stream.reshape(JWC // 16, 16).T.astype(np.int16))
                cd = np.bincount(dstc_s[s0:s1], minlength=CHUNK_NODES[c])
                b = np.cumsum(cd)
                blist = np.full(NBC, b[-1], np.int64)
                blist[0] = 0
                blist[1:1 + CHUNK_NODES[c]] = b
                bidx[16 * g:16 * (g + 1)] = (
                    blist.reshape(NBC // 16, 16).T.astype(np.int16))
            tabs_all[k][c] = tab
            gbs_all[k][c] = np.concatenate([gidx, bidx], axis=1)

    return {
        "NECS": NECS,
        "JWCS": JWCS,
        "tabs": tabs_all,
        "gbs": gbs_all,
        "aug": aug_cores,
        "papt": papt_cores,
        "cnt": cnt,
    }


def _head(G, cnt, inputs):
    f = np.float32
    W2 = np.asarray(inputs["W2"], f)
    b2 = np.asarray(inputs["b2"], f)
    Wg = np.asarray(inputs["Wg"], f)
    bg = np.asarray(inputs["bg"], f)
    Et = np.asarray(inputs["Et"], f)
    Ek = np.asarray(inputs["Ek"], f)
    Ev = np.asarray(inputs["Ev"], f)
    Wp = np.asarray(inputs["Wp"], f)
    bp = np.asarray(inputs["bp"], f)
    Ekid = np.asarray(inputs["Ekid"], f)
    Wc = np.asarray(inputs["Wc"], f)
    bc = np.asarray(inputs["bc"], f)
    Wl = np.asarray(inputs["Wl"], f)
    bl = np.asarray(inputs["bl"], f)
    Wm1 = np.asarray(inputs["Wm1"], f)
    bm1 = np.asarray(inputs["bm1"], f)
    Wm2 = np.asarray(inputs["Wm2"], f)
    bm2 = np.asarray(inputs["bm2"], f)
    st = np.asarray(inputs["sol_type_idx"], np.int64)
    sk = np.asarray(inputs["sol_key_idx"], np.int64)
    sv = np.asarray(inputs["sol_val_idx"], np.int64)
    kid = np.asarray(inputs["kernel_id"], np.int64)
    cond = np.asarray(inputs["cond_vec"], f)
    loc = np.asarray(inputs["local_feats"], f)

    relu = lambda a: np.maximum(a, 0.0).astype(f)

    Ph2 = G[:B] @ W2 + cnt[:, None] * b2[None, :] + G[B:]
    g = (Ph2 / np.maximum(cnt, 1.0)[:, None]) @ Wg + bg

    seq_mean = np.concatenate(
        [Et[st].mean(axis=1), Ek[sk].mean(axis=1), Ev[sv].mean(axis=1)], axis=-1
    ).astype(f)
    p = relu(seq_mean @ Wp + bp)
    kvec = Ekid[kid]
    c = relu(cond @ Wc + bc)
    l = relu(loc @ Wl + bl)
    xf = np.concatenate([g, p, kvec, c, l], axis=1).astype(f)
    return (relu(xf @ Wm1 + bm1) @ Wm2 + bm2).astype(f)


def kernel(**inputs) -> np.ndarray:
    import ml_dtypes
    from concourse.bass_utils import run_bass_kernel_spmd

    bf = ml_dtypes.bfloat16

    pre = _preprocess(inputs["x"], inputs["edge_index"], inputs["batch_idx"])
    shape_key = (pre["NECS"], pre["JWCS"])
    if shape_key not in _compiled:
        _compiled[shape_key] = _build_nc(shape_key)
    nc = _compiled[shape_key]

    W1 = np.asarray(inputs["W1"], np.float32)
    b1 = np.asarray(inputs["b1"], np.float32)
    w1a = np.concatenate([W1, b1[None, :]], axis=0).astype(bf)       # [7,H]
    wp = np.zeros((P128, H), np.float32)                             # selp @ W1
    for g in range(NG):
        wp[16 * g:16 * g + 6] = W1
    wp = wp.astype(bf)

    in_maps = []
    for k in range(NG):
        m = {
            "aug7": pre["aug"][k],
            "w1a": w1a,
            "wp": wp,
            "papt": pre["papt"][k],
        }
        for c in range(NCHUNK):
            m[f"tab{c}"] = pre["tabs"][k][c]
            m[f"gb{c}"] = pre["gbs"][k][c]
        in_maps.append(m)

    res = run_bass_kernel_spmd(nc, in_maps, core_ids=list(range(NG)))
    G = np.zeros((P128, P128), np.float64)
    for r in res.results:
        G += r["gout"].astype(np.float64)
    G = G.astype(np.float32)

    return _head(G, pre["cnt"], inputs)


# revision 24
# speedup vs baseline: 1.0139x; 1.0139x over previous
"""Trainium2 Bass kernel for nn_CryptoGNN (2-layer GCN + pooled heads).

Math (validated against the reference):
  With A = normalized adjacency (incl. self loops), P = [B,N] pooling matrix:
    r_i = sum_{real e: j->i} dis_j x_j + dis_i x_i          (6 feats)
    z_i = r_i @ W1 + sqrt(deg_i) b1        ->  h1_i = dis_i * relu(z_i)
    G   = [PA; P] @ h1   ([128,128]; PA = P@A dense, built on host)
  Host head finishes:  P@h2 = (PA@h1)@W2 + cnt*b2 + P@h1, then the tiny
  [64,*] MLPs (microseconds, numpy).

Sharding: nodes split into 8 contiguous dst-shards of 12544, one NeuronCore
each.  No collectives: the host sums the 8 partial [128,128] G outputs.

Per-core device pipeline (4 dst-chunks, sized small/big/big/small so the
first table DMA and the last chunk's tail are short):
  * per (chunk, src-group) compacted feature table [128, NEC_c] f32 -- only
    src columns with >=1 edge into the (core, chunk) are shipped (~20% of
    12.5k), dead rows host-zeroed, dis[src] pre-folded.
  * GPSIMD ap_gather #1: per-group dst-sorted edge stream of src columns
  * fp32 prefix scan along the stream (DVE tensor_tensor_scan)
  * GPSIMD ap_gather #2 of per-dst boundary prefix values
  * DVE shifted difference -> per-dst segment sums, downcast to bf16 dt
    (emitted in 1024-col pieces so mm batches start early)
  * mm1 (bf16): z tile = dt[:,tile]^T @ (selp@W1) + aug7[:,tile]^T @ [W1;b1]
    -- the 8-group fold is pre-composed into the host weight wp, so there
    is no separate fold stage or PSUM copy.  aug7 rows = dis*x self-loop
    feats + sqrt(deg) (for b1), host-computed.
  * relu in 8-tile batches (Act) -> bf16 h1
  * mm2 (bf16): G += papt[:,tile]^T @ h1; papt is bf16 with dis[dst]
    pre-folded, stored pair-interleaved ([49,128,256]) so its DMA runs at
    full rate (512B contiguous runs).

GPSIMD is the bottleneck engine (~32us busy); the 4-chunk structure keeps
it streaming continuously while DVE scans/diffs and PE/Act run earlier
chunks' matmuls.  Index tiles keep the boundary half 4-byte aligned
(JWC_c % 32 == 0) -- the gather ucode requires it (CoreSim does not
model this; misalignment silently corrupts on silicon).
"""

import sys

if "/opt/trn_rl_repo" not in sys.path:
    sys.path.insert(0, "/opt/trn_rl_repo")

import numpy as np

N = 100000
E = 600000
B = 64
IN = 6
H = 128

NSHARD = 12544            # nodes per core shard
NG = 8                    # src groups (= table partition groups)
NPAD = NSHARD * NG        # 100352
NT = NSHARD // 128        # 98 node tiles per shard
P128 = 128

NCHUNK = 4
CHUNK_TILES = (12, 32, 32, 22)
CHUNK_NODES = tuple(t * 128 for t in CHUNK_TILES)          # 1536 4096 4096 2816
CHUNK_OFF = (0, 1536, 5632, 9728)
CHUNK_TILE0 = (0, 12, 44, 76)
NBCS = tuple(((n + 1 + 15) // 16) * 16 for n in CHUNK_NODES)

_compiled = {}


def _r16(v):
    return ((int(v) + 15) // 16) * 16


def _r32(v):
    # multiples of 32 keep the bidx half of the concatenated int16 index
    # tile 4-byte aligned for the GPSIMD gather ucode
    return ((int(v) + 31) // 32) * 32


def _build_nc(shape_key):
    import concourse.bacc as bacc
    import concourse.mybir as mybir
    from concourse import tile

    NECS, JWCS = shape_key
    f32 = mybir.dt.float32
    bf16 = mybir.dt.bfloat16
    i16 = mybir.dt.int16

    NEC_MX = max(NECS)
    JWC_MX = max(JWCS)
    NBC_MX = max(NBCS)

    nc = bacc.Bacc("TRN2", target_bir_lowering=False, debug=False)

    tabs = [nc.declare_dram_parameter(f"tab{c}", [P128, NECS[c]], f32,
                                      isOutput=False) for c in range(NCHUNK)]
    gbs = [nc.declare_dram_parameter(f"gb{c}", [P128, (JWCS[c] + NBCS[c]) // 16],
                                     i16, isOutput=False) for c in range(NCHUNK)]
    aug7 = nc.declare_dram_parameter("aug7", [7, NSHARD], bf16, isOutput=False)
    w1a = nc.declare_dram_parameter("w1a", [7, H], bf16, isOutput=False)
    wp = nc.declare_dram_parameter("wp", [P128, H], bf16, isOutput=False)
    papt = nc.declare_dram_parameter("papt", [NT // 2, P128, 256], bf16,
                                     isOutput=False)
    gout = nc.declare_dram_parameter("gout", [P128, P128], f32, isOutput=True)

    with tile.TileContext(nc) as tc:
        with (
            tc.tile_pool(name="small", bufs=1) as small,
            tc.tile_pool(name="tabp", bufs=3) as tabp,
            tc.tile_pool(name="idxp", bufs=4) as idxp,
            tc.tile_pool(name="gathp", bufs=3) as gathp,
            tc.tile_pool(name="bndp", bufs=2) as bndp,
            tc.tile_pool(name="dtp", bufs=2) as dtp,
            tc.tile_pool(name="h1p", bufs=3) as h1p,
            tc.tile_pool(name="ps1", bufs=2, space="PSUM") as ps1p,
            tc.tile_pool(name="psG", bufs=1, space="PSUM") as psGp,
        ):
            # ---- constants (DMAs issued later, after the first tables) ----
            # wp = selp @ W1 (host-precomposed group-replicated W1 rows) so
            # mm1 contracts the bf16 diff output directly -- no fold stage.
            wp_t = small.tile([P128, H], bf16)
            w1_t = small.tile([7, H], bf16)
            # aug rows: 0-5 dis*x self-loop feats, 6 sqrt(deg)
            axT = small.tile([7, NSHARD], bf16)

            def dma_consts():
                nc.sync.dma_start(out=wp_t[:], in_=wp[:])
                nc.sync.dma_start(out=w1_t[:], in_=w1a[:])
                nc.sync.dma_start(out=axT[:], in_=aug7[:])

            # whole papt in SBUF (bf16, interleaved pairs), 4 slab DMAs
            papt_sb = small.tile([P128, NT // 2 * 256], bf16)
            SLABS = [(0, 13), (13, 12), (25, 12), (37, 12)]

            G_ps = psGp.tile([P128, P128], f32, tag="G")

            tab_t = [None] * NCHUNK
            dt_t = [None] * NCHUNK
            gb_t = [None] * NCHUNK
            gath_t = [None] * NCHUNK
            bnd_t = [None] * NCHUNK

            def dma_tab(c):
                tab_t[c] = tabp.tile([P128, NEC_MX], f32, tag="tab",
                                     name=f"tab_t{c}")
                nc.sync.dma_start(out=tab_t[c][:, 0:NECS[c]], in_=tabs[c][:])
                gb_t[c] = idxp.tile([P128, (JWCS[c] + NBCS[c]) // 16], i16,
                                    tag=f"gb{c}", name=f"gb_t{c}")
                nc.sync.dma_start(out=gb_t[c][:], in_=gbs[c][:])

            def dma_slab(s):
                u0, nu = SLABS[s]
                nc.sync.dma_start(
                    out=papt_sb[:, u0 * 256:(u0 + nu) * 256].rearrange(
                        "p (u j) -> p u j", j=256),
                    in_=papt[u0:u0 + nu].rearrange("u p j -> p u j"),
                )

            def g1(c):
                gath_t[c] = gathp.tile([P128, JWC_MX], f32, tag="gath",
                                       name=f"gath_t{c}")
                nc.gpsimd.ap_gather(
                    out_ap=gath_t[c][:, 0:JWCS[c]],
                    in_ap=tab_t[c][:, 0:NECS[c]],
                    idxs_ap=gb_t[c][:, :JWCS[c] // 16],
                    channels=P128, num_elems=NECS[c], d=1, num_idxs=JWCS[c],
                )

            def scan(c):
                g = gath_t[c][:, 0:JWCS[c]]
                nc.vector.tensor_tensor_scan(
                    out=g, data0=g, data1=g, initial=0.0,
                    op0=mybir.AluOpType.add, op1=mybir.AluOpType.bypass,
                )

            def g2(c):
                bnd_t[c] = bndp.tile([P128, NBC_MX], f32, tag="bnd",
                                     name=f"bnd_t{c}")
                nc.gpsimd.ap_gather(
                    out_ap=bnd_t[c][:, 0:NBCS[c]],
                    in_ap=gath_t[c][:, 0:JWCS[c]],
                    idxs_ap=gb_t[c][:, JWCS[c] // 16:],
                    channels=P128, num_elems=JWCS[c], d=1, num_idxs=NBCS[c],
                )

            def diff(c):
                # dt = bnd[:,1:] - bnd[:,:-1] downcast to bf16, emitted in
                # 1024-col pieces so the first mm batch starts early
                cn = CHUNK_NODES[c]
                step = 1024
                dt_t[c] = dtp.tile([P128, max(CHUNK_NODES)], bf16, tag="dt",
                                   name=f"dt_t{c}")
                for p0 in range(0, cn, step):
                    sz = min(step, cn - p0)
                    nc.vector.tensor_tensor(
                        out=dt_t[c][:, p0:p0 + sz],
                        in0=bnd_t[c][:, p0 + 1:p0 + 1 + sz],
                        in1=bnd_t[c][:, p0:p0 + sz],
                        op=mybir.AluOpType.subtract,
                    )

            pending = []   # (h1_t, first_tile, ntiles) awaiting mm2

            def flush_mm2():
                h1_t, t0, m = pending.pop(0)
                for u in range(m):
                    t = t0 + u
                    uu, vv = divmod(t, 2)
                    pcol = uu * 256 + vv * 128
                    nc.tensor.matmul(
                        out=G_ps[:],
                        lhsT=papt_sb[:, pcol:pcol + 128],
                        rhs=h1_t[:, u * H:(u + 1) * H],
                        start=(t == 0), stop=(t == NT - 1),
                    )

            def mm(c):
                # z tile = dt[:,tile]^T @ wp + aug[:,tile]^T @ [W1;b1]
                # (PSUM accumulate); relu in 8-tile batches (alternating
                # Act/DVE); mm2 is software-pipelined one batch behind so
                # the in-order PE queue never stalls on a relu.
                off = CHUNK_OFF[c]
                dt = dt_t[c]
                tc0 = CHUNK_TILE0[c]
                qb = 8
                for q in range(0, CHUNK_TILES[c], qb):
                    m = min(qb, CHUNK_TILES[c] - q)
                    ps1_t = ps1p.tile([P128, 8 * H], f32, tag="ps1")
                    for u in range(m):
                        lo = (q + u) * 128
                        nc.tensor.matmul(
                            out=ps1_t[:, u * H:(u + 1) * H],
                            lhsT=dt[:, lo:lo + 128],
                            rhs=wp_t[:],
                            start=True, stop=False,
                        )
                        nc.tensor.matmul(
                            out=ps1_t[:, u * H:(u + 1) * H],
                            lhsT=axT[:, off + lo:off + lo + 128],
                            rhs=w1_t[:],
                            start=False, stop=True,
                        )
                    while pending:
                        flush_mm2()
                    h1_t = h1p.tile([P128, 8 * H], bf16, tag="h1")
                    nc.scalar.activation(
                        out=h1_t[:, :m * H], in_=ps1_t[:, :m * H],
                        func=mybir.ActivationFunctionType.Relu)
                    pending.append((h1_t, tc0 + q, m))

            # ---- pipelined emission (deps drive the schedule; tables
            # first so GPSIMD starts early and never starves) ----
            dma_tab(0)
            dma_tab(1)
            dma_consts()
            g1(0)
            dma_tab(2)
            scan(0)
            g1(1)
            dma_tab(3)
            g2(0)
            scan(1)
            diff(0)
            dma_slab(0)
            mm(0)
            g1(2)
            dma_slab(1)
            g2(1)
            scan(2)
            diff(1)
            mm(1)
            g1(3)
            dma_slab(2)
            dma_slab(3)
            g2(2)
            scan(3)
            diff(2)
            mm(2)
            g2(3)
            diff(3)
            mm(3)
            while pending:
                flush_mm2()

            G_sb = small.tile([P128, P128], f32)
            nc.scalar.activation(
                out=G_sb[:], in_=G_ps[:],
                func=mybir.ActivationFunctionType.Copy,
            )
            nc.sync.dma_start(out=gout[:], in_=G_sb[:])

    nc.compile()
    return nc


def _preprocess(x, edge_index, batch_idx):
    """Host-side integer/structure preprocessing -> per-core input maps
    (minus weights) + head constants."""
    import ml_dtypes

    bf = ml_dtypes.bfloat16

    src = np.asarray(edge_index[0], dtype=np.int64)
    dst = np.asarray(edge_index[1], dtype=np.int64)

    deg = (np.bincount(dst, minlength=N) + 1.0).astype(np.float32)
    dis = (1.0 / np.sqrt(deg)).astype(np.float32)
    sqdeg = np.sqrt(deg).astype(np.float32)

    bi = np.asarray(batch_idx, dtype=np.int64)
    cnt = np.bincount(bi, minlength=B).astype(np.float32)

    dis_pad = np.zeros(NPAD, np.float32)
    dis_pad[:N] = dis

    # dense PA = P @ A  [B, NPAD] (incl. self loops), exact fp64 accumulate
    loop = np.arange(N, dtype=np.int64)
    src2 = np.concatenate([src, loop])
    dst2 = np.concatenate([dst, loop])
    w = (dis[src2] * dis[dst2]).astype(np.float64)
    flat = bi[dst2] * NPAD + src2
    PA = np.bincount(flat, weights=w, minlength=B * NPAD)
    PA = PA.reshape(B, NPAD).astype(np.float32)
    Pm = np.zeros((B, NPAD), np.float32)
    Pm[bi, np.arange(N)] = 1.0
    papt_full = (np.concatenate([PA, Pm], axis=0)
                 * dis_pad[None, :]).T.copy()      # [NPAD, 128]

    # per-core papt: bf16, DMA-interleaved [49, 128, 256]
    papt_cores = []
    for k in range(NG):
        pk = papt_full[k * NSHARD:(k + 1) * NSHARD].astype(bf)
        pk = pk.reshape(NT // 2, 2, 128, 128).transpose(0, 2, 1, 3)
        papt_cores.append(np.ascontiguousarray(pk.reshape(NT // 2, 128, 256)))

    # per-core aug rows: 0-5 dis*x, 6 sqrt(deg)
    x_np = np.asarray(x, dtype=np.float32)
    selfx = (x_np * dis[:, None]).astype(np.float32)   # [N, 6]
    aug_cores = []
    for k in range(NG):
        a = np.zeros((7, NSHARD), np.float32)
        n0 = k * NSHARD
        n1 = min(n0 + NSHARD, N)
        a[0:6, 0:n1 - n0] = selfx[n0:n1].T
        a[6, 0:n1 - n0] = sqdeg[n0:n1]
        aug_cores.append(a.astype(bf))

    # ---- edge partitioning: (core, chunk, group), dst-sorted ----
    core = dst // NSHARD
    dstl = dst - core * NSHARD
    ch_off = np.asarray(CHUNK_OFF + (NSHARD,), np.int64)
    chunk = np.searchsorted(ch_off, dstl, side="right") - 1
    dstc = dstl - ch_off[chunk]
    grp = src // NSHARD
    srcl = src - grp * NSHARD

    cell = (core * NCHUNK + chunk) * NG + grp          # [0, 256)
    key = cell * 8192 + dstc                           # dstc < 8192
    order = np.argsort(key, kind="stable")
    cell_s = cell[order]
    srcl_s = srcl[order]
    dstc_s = dstc[order]
    counts = np.bincount(cell_s, minlength=NG * NCHUNK * NG)
    starts = np.zeros(NG * NCHUNK * NG + 1, np.int64)
    np.cumsum(counts, out=starts[1:])

    # pass 1: per-cell compaction
    uniqs = {}
    invs = {}
    max_used = [0] * NCHUNK
    max_cnt = [0] * NCHUNK
    for k in range(NG):
        for c in range(NCHUNK):
            for g in range(NG):
                ci = (k * NCHUNK + c) * NG + g
                s0, s1 = starts[ci], starts[ci + 1]
                u, inv = np.unique(srcl_s[s0:s1], return_inverse=True)
                uniqs[(k, c, g)] = u
                invs[(k, c, g)] = inv
                max_used[c] = max(max_used[c], len(u))
                max_cnt[c] = max(max_cnt[c], s1 - s0)

    NECS = tuple(_r16(mu + 16) for mu in max_used)     # last col(s) stay zero
    JWCS = tuple(_r32(mc + 2) for mc in max_cnt)

    tabs_all = [[None] * NCHUNK for _ in range(NG)]
    gbs_all = [[None] * NCHUNK for _ in range(NG)]
    for k in range(NG):
        for c in range(NCHUNK):
            NEC, JWC, NBC = NECS[c], JWCS[c], NBCS[c]
            zcol = NEC - 1
            tab = np.zeros((P128, NEC), np.float32)
            gidx = np.full((P128, JWC // 16), zcol, np.int16)
            bidx = np.zeros((P128, NBC // 16), np.int16)
            for g in range(NG):
                ci = (k * NCHUNK + c) * NG + g
                s0, s1 = starts[ci], starts[ci + 1]
                u = uniqs[(k, c, g)]
                inv = invs[(k, c, g)]
                nu = len(u)
                if nu:
                    gl = g * NSHARD + u
                    tab[16 * g:16 * g + 6, :nu] = (x_np[gl] * dis[gl, None]).T
                stream = np.full(JWC, zcol, np.int64)
                stream[1:1 + (s1 - s0)] = inv
                gidx[16 * g:16 * (g + 1)] = (
                    stream.reshape(JWC // 16, 16).T.astype(np.int16))
                cd = np.bincount(dstc_s[s0:s1], minlength=CHUNK_NODES[c])
                b = np.cumsum(cd)
                blist = np.full(NBC, b[-1], np.int64)
                blist[0] = 0
                blist[1:1 + CHUNK_NODES[c]] = b
                bidx[16 * g:16 * (g + 1)] = (
                    blist.reshape(NBC // 16, 16).T.astype(np.int16))
            tabs_all[k][c] = tab
            gbs_all[k][c] = np.concatenate([gidx, bidx], axis=1)

    return {
        "NECS": NECS,
        "JWCS": JWCS,
        "tabs": tabs_all,
        "gbs": gbs_all,
        "aug": aug_cores,
        "papt": papt_cores,
        "cnt": cnt,
    }


def _head(G, cnt, inputs):
    f = np.float32
    W2 = np.asarray(inputs["W2"], f)
    b2 = np.asarray(inputs["b2"], f)
    Wg = np.asarray(inputs["Wg"], f)
    bg = np.asarray(inputs["bg"], f)
    Et = np.asarray(inputs["Et"], f)
    Ek = np.asarray(inputs["Ek"], f)
    Ev = np.asarray(inputs["Ev"], f)
    Wp = np.asarray(inputs["Wp"], f)
    bp = np.asarray(inputs["bp"], f)
    Ekid = np.asarray(inputs["Ekid"], f)
    Wc = np.asarray(inputs["Wc"], f)
    bc = np.asarray(inputs["bc"], f)
    Wl = np.asarray(inputs["Wl"], f)
    bl = np.asarray(inputs["bl"], f)
    Wm1 = np.asarray(inputs["Wm1"], f)
    bm1 = np.asarray(inputs["bm1"], f)
    Wm2 = np.asarray(inputs["Wm2"], f)
    bm2 = np.asarray(inputs["bm2"], f)
    st = np.asarray(inputs["sol_type_idx"], np.int64)
    sk = np.asarray(inputs["sol_key_idx"], np.int64)
    sv = np.asarray(inputs["sol_val_idx"], np.int64)
    kid = np.asarray(inputs["kernel_id"], np.int64)
    cond = np.asarray(inputs["cond_vec"], f)
    loc = np.asarray(inputs["local_feats"], f)

    relu = lambda a: np.maximum(a, 0.0).astype(f)

    Ph2 = G[:B] @ W2 + cnt[:, None] * b2[None, :] + G[B:]
    g = (Ph2 / np.maximum(cnt, 1.0)[:, None]) @ Wg + bg

    seq_mean = np.concatenate(
        [Et[st].mean(axis=1), Ek[sk].mean(axis=1), Ev[sv].mean(axis=1)], axis=-1
    ).astype(f)
    p = relu(seq_mean @ Wp + bp)
    kvec = Ekid[kid]
    c = relu(cond @ Wc + bc)
    l = relu(loc @ Wl + bl)
    xf = np.concatenate([g, p, kvec, c, l], axis=1).astype(f)
    return (relu(xf @ Wm1 + bm1) @ Wm2 + bm2).astype(f)


def kernel(**inputs) -> np.ndarray:
    import ml_dtypes
    from concourse.bass_utils import run_bass_kernel_spmd

    bf = ml_dtypes.bfloat16

    pre = _preprocess(inputs["x"], inputs["edge_index"], inputs["batch_idx"])
    shape_key = (pre["NECS"], pre["JWCS"])
    if shape_key not in _compiled:
        _compiled[shape_key] = _build_nc(shape_key)
    nc = _compiled[shape_key]

    W1 = np.asarray(inputs["W1"], np.float32)
    b1 = np.asarray(inputs["b1"], np.float32)
    w1a = np.concatenate([W1, b1[None, :]], axis=0).astype(bf)       # [7,H]
    wp = np.zeros((P128, H), np.float32)                             # selp @ W1
    for g in range(NG):
        wp[16 * g:16 * g + 6] = W1
    wp = wp.astype(bf)

    in_maps = []
    for k in range(NG):
        m = {
            "aug7": pre["aug"][k],
            "w1a": w1a,
            "wp": wp,
            "papt": pre["papt"][k],
        }
        for c in range(NCHUNK):
            m[f"tab{c}"] = pre["tabs"][k][c]
            m[f"gb{c}"] = pre["gbs"][k][c]
        in_maps.append(m)

    res = run_bass_kernel_spmd(nc, in_maps, core_ids=list(range(NG)))
    G = np.zeros((P128, P128), np.float64)
    for r in res.results:
        G += r["gout"].astype(np.float64)
    G = G.astype(np.float32)

    return _head(G, pre["cnt"], inputs)
